# revision 1
# baseline (speedup 1.0000x reference)
"""CosfacePairwiseLoss Trainium2 kernel (8 NeuronCores, Bass/Tile).

Strategy:
- Loss is invariant under joint row/col permutation of the pairwise matrix, so
  the host sorts rows by label; each row's positives then live in a narrow
  diagonal band (max group size ~25 for the 1000-label/8192-row regime).
- Rows are sharded 1024/core. Each core normalizes its rows (bf16), AllGathers
  the normalized features, and computes its [1024, 8192] similarity block with
  bf16 matmuls (f32 PSUM).
- Dense logsumexp(neg): host supplies a 0/1 positives mask (bf16); one fused
  DVE op computes 30*sim - 1000*mask from PSUM, ACT exp accumulates row sums.
  Positives underflow to exactly 0, matching the reference's -10-fill behavior
  in f32.
- logsumexp(pos): a small [128, 256] band matmul per row-tile (dynamic rhs
  offset = 1024*core + 128*m) + the same host mask slice gives
  sum(exp(9 - 30*sim)) over the group.
- Per-row losses go back to the host, which averages (the only unsharded op).
"""
import os
import numpy as np
import ml_dtypes

import concourse.bass as bass
import concourse.bacc as bacc
import concourse.mybir as mybir
import concourse.tile as tile
from concourse.bass_utils import run_bass_kernel_spmd

F32 = mybir.dt.float32
BF16 = mybir.dt.bfloat16
AF = mybir.ActivationFunctionType
ALU = mybir.AluOpType

N, D, NCORES = 8192, 512, 8
R = N // NCORES  # rows per core
MT = R // 128  # row-tiles per core
NCH = N // 512  # 512-wide column chunks
W = 256  # band window width
PAD = 64  # fT padding each side
NP_ = N + 2 * PAD  # padded columns

_CACHED = {}


def _build_nc(sim_variant: bool = False, skip_epi: bool = False, split_tpose: int = 1):
    n_dev = 1 if sim_variant else NCORES
    nc = bacc.Bacc("TRN2", target_bir_lowering=False, debug=False, num_devices=n_dev)

    feat_in = nc.dram_tensor("feat_in", [R, D], F32, kind="ExternalInput").ap()
    mask_in = nc.dram_tensor("mask_in", [R, NP_], BF16, kind="ExternalInput").ap()
    o_loss = nc.dram_tensor("o_loss", [128, MT], F32, kind="ExternalOutput").ap()

    with tile.TileContext(nc) as tc:
        with (
            tc.tile_pool(name="io", bufs=3) as io,
            tc.tile_pool(name="fbp", bufs=3) as fbp,
            tc.tile_pool(name="stats", bufs=8) as stats,
            tc.tile_pool(name="singles", bufs=1) as singles,
            tc.tile_pool(name="ftp", bufs=1) as ftp,
            tc.tile_pool(name="maskp", bufs=2) as maskp,
            tc.tile_pool(name="up", bufs=3) as upool,
            tc.tile_pool(name="ep", bufs=3) as epool,
            tc.tile_pool(name="bsmall", bufs=2) as bsmall,
            tc.tile_pool(name="nsp", bufs=2) as nsp,
            tc.tile_pool(name="psmain", bufs=6, space="PSUM") as psmain,
            tc.tile_pool(name="psband", bufs=2, space="PSUM") as psband,
            tc.tile_pool(name="dram", bufs=1, space="DRAM") as dram,
        ):
            cc_in = dram.tile([R, D], BF16)
            cc_out = dram.tile([N, D], BF16, addr_space="Shared")

            bias150 = singles.tile([128, 1], F32)
            nc.vector.memset(bias150, -150.0)
            losses = singles.tile([128, MT], F32)

            # ---- Phase A: normalize own rows -> bf16, stage to DRAM ----
            for m in range(MT):
                x = io.tile([128, D], F32, tag="x")
                nc.sync.dma_start(out=x, in_=feat_in[bass.ts(m, 128), :])
                scr = io.tile([128, D], F32, tag="scr")
                ss = stats.tile([128, 1], F32, tag="ss")
                nc.scalar.activation(scr, x, AF.Square, accum_out=ss)
                ssc = stats.tile([128, 1], F32, tag="ssc")
                nc.vector.tensor_scalar_max(ssc, ss, 1e-16)
                lnss = stats.tile([128, 1], F32, tag="lnss")
                nc.scalar.activation(lnss, ssc, AF.Ln)
                rinv = stats.tile([128, 1], F32, tag="rinv")
                nc.scalar.activation(rinv, lnss, AF.Exp, scale=-0.5)
                fb = fbp.tile([128, D], BF16, tag="fb")
                nc.vector.tensor_scalar_mul(fb, x, rinv)
                nc.sync.dma_start(out=cc_in[bass.ts(m, 128), :], in_=fb)

            # ---- own-block fT (static lhsT source) ----
            ft_own = [singles.tile([128, R], BF16, name=f"ft_own{k}") for k in range(4)]
            for k in range(4):
                nc.sync.dma_start_transpose(
                    out=ft_own[k], in_=cc_in[:, bass.ts(k, 128)]
                )

            # ---- AllGather normalized features ----
            if sim_variant:
                nc.sync.dma_start(out=cc_out[0:R, :], in_=cc_in[:, :])
            else:
                nc.gpsimd.collective_compute(
                    "AllGather",
                    ALU.bypass,
                    replica_groups=[list(range(NCORES))],
                    ins=[cc_in.opt()],
                    outs=[cc_out.opt()],
                )

            # ---- fT_all (padded) ----
            ftall = [singles.tile([128, NP_], BF16, name=f"ftall{k}") for k in range(4)]
            for k in range(4):
                nc.vector.memset(ftall[k][:, 0:PAD], 0.0)
                nc.vector.memset(ftall[k][:, NP_ - PAD : NP_], 0.0)
                piece = N // split_tpose
                for t in range(split_tpose):
                    nc.sync.dma_start_transpose(
                        out=ftall[k][:, PAD + t * piece : PAD + (t + 1) * piece],
                        in_=cc_out[t * piece : (t + 1) * piece, bass.ts(k, 128)],
                    )

            pid_pe = nc.tensor.partition_id()
            pid_dve = nc.vector.partition_id()

            # ---- Phase C: per row-tile ----
            CHUNK_GROUPS = [list(range(0, 6)), list(range(6, 12)), list(range(12, 16))]
            for m in range(MT):
                mt_t = maskp.tile([128, NP_], BF16, tag="mask")
                nc.sync.dma_start(out=mt_t, in_=mask_in[bass.ts(m, 128), :])
                nsum = nsp.tile([128, NCH], F32, tag="nsum")

                psums = {}
                for grp in CHUNK_GROUPS:
                    for k in range(4):
                        for n in grp:
                            if k == 0:
                                psums[n] = psmain.tile([128, 512], F32, tag="ps", name=f"ps{n}")
                            nc.tensor.matmul(
                                psums[n],
                                ft_own[k][:, bass.ts(m, 128)],
                                ftall[k][:, PAD + 512 * n : PAD + 512 * (n + 1)],
                                start=(k == 0),
                                stop=(k == 3),
                            )
                    for n in grp:
                        if skip_epi:
                            u = upool.tile([128, 512], F32, tag="u")
                            nc.vector.tensor_copy(u[:, 0:8], psums[n][:, 0:8])
                            continue
                        u = upool.tile([128, 512], F32, tag="u")
                        nc.vector.scalar_tensor_tensor(
                            u,
                            in0=mt_t[:, PAD + 512 * n : PAD + 512 * (n + 1)],
                            scalar=-33.333333,
                            in1=psums[n],
                            op0=ALU.mult,
                            op1=ALU.add,
                        )
                        e = epool.tile([128, 512], F32, tag="e")
                        nc.scalar.activation(
                            e, u, AF.Exp, scale=30.0, accum_out=nsum[:, n : n + 1]
                        )

                # band (positives) pass
                if skip_epi:
                    nc.vector.memset(losses[:, m : m + 1], 0.0)
                    continue
                off_pe = pid_pe * 1024 + 128 * m
                off_dve = pid_dve * 1024 + 128 * m
                bp = psband.tile([128, W], F32, tag="bps")
                for k in range(4):
                    nc.tensor.matmul(
                        bp,
                        ft_own[k][:, bass.ts(m, 128)],
                        ftall[k][:, bass.ds(off_pe, W)],
                        start=(k == 0),
                        stop=(k == 3),
                    )
                ub = bsmall.tile([128, W], F32, tag="ub")
                nc.vector.scalar_tensor_tensor(
                    ub,
                    in0=mt_t[:, bass.ds(off_dve, W)],
                    scalar=5.3,
                    in1=bp,
                    op0=ALU.mult,
                    op1=ALU.subtract,
                )
                eb = bsmall.tile([128, W], F32, tag="eb")
                pcol = stats.tile([128, 1], F32, tag="pcol")
                nc.scalar.activation(
                    eb, ub, AF.Exp, scale=30.0, bias=bias150, accum_out=pcol
                )

                # combine: loss = softplus(ln(P) + ln(N))
                ncol = stats.tile([128, 1], F32, tag="ncol")
                nc.vector.reduce_sum(ncol, nsum, axis=mybir.AxisListType.X)
                lp = stats.tile([128, 1], F32, tag="lp")
                nc.scalar.activation(lp, pcol, AF.Ln)
                lnn = stats.tile([128, 1], F32, tag="lnn")
                nc.scalar.activation(lnn, ncol, AF.Ln)
                xr = stats.tile([128, 1], F32, tag="xr")
                nc.vector.tensor_tensor(xr, lp, lnn, op=ALU.add)
                er = stats.tile([128, 1], F32, tag="er")
                nc.scalar.activation(er, xr, AF.Exp)
                er1 = stats.tile([128, 1], F32, tag="er1")
                nc.vector.tensor_scalar_add(er1, er, 1.0)
                nc.scalar.activation(losses[:, m : m + 1], er1, AF.Ln)

            nc.sync.dma_start(out=o_loss, in_=losses)

    nc.compile()
    return nc


def kernel(feat: np.ndarray, label: np.ndarray) -> np.ndarray:
    feat = np.asarray(feat, dtype=np.float32)
    label = np.asarray(label)
    assert feat.shape == (N, D) and label.shape == (N,)

    # sort rows by label (loss is permutation invariant)
    perm = np.argsort(label, kind="stable")
    lab_s = np.asarray(label)[perm]
    feat_s = feat[perm]

    # group bounds per row
    lab64 = lab_s.astype(np.int64)
    starts = np.searchsorted(lab64, lab64, side="left")
    ends = np.searchsorted(lab64, lab64, side="right")

    # verify every row's group fits its tile's band window
    rows = np.arange(N)
    tile_of = rows // 128
    woff = tile_of * 128 - PAD  # window [woff, woff + W)
    assert (starts >= woff).all() and (ends <= woff + W).all(), (
        "label group exceeds band window; widen W"
    )

    in_maps = []
    for r in range(NCORES):
        sl = slice(r * R, (r + 1) * R)
        rl = lab64[sl][:, None]
        mask_rows = (rl == lab64[None, :]).astype(ml_dtypes.bfloat16)
        maskp = np.zeros((R, NP_), dtype=ml_dtypes.bfloat16)
        maskp[:, PAD : PAD + N] = mask_rows
        in_maps.append({"feat_in": feat_s[sl], "mask_in": maskp})

    if "nc" not in _CACHED:
        _CACHED["nc"] = _build_nc()
    nc = _CACHED["nc"]

    res = run_bass_kernel_spmd(nc, in_maps, core_ids=list(range(NCORES)))
    loss_rows = np.concatenate(
        [res.results[r]["o_loss"].T.reshape(-1) for r in range(NCORES)]
    )
    return np.float32(loss_rows.mean())



# revision 2
# speedup vs baseline: 2.0099x; 2.0099x over previous
"""CosfacePairwiseLoss Trainium2 kernel (8 NeuronCores, Bass/Tile).

Strategy:
- Host sorts rows by label and bin-packs whole label groups into 8 bins of
  exactly 1024 rows (FFD), so no group straddles a core boundary. Each row's
  positives then live inside its OWN core's 1024 columns, within a static
  256-wide window per 128-row tile.
- Each core normalizes its rows (bf16) and transposes them on the PE
  (identity matmul from a host-supplied identity, bf16 PSUM) into
  ft_own [4][128,1024] — no DMA transposes (xbar-slow) and no GPSIMD
  affine_select (~70 ms/exec on this runtime).
- The TRANSPOSED block [512,1024] is AllGathered (1 MB -> 8 MB); fT tiles of
  all cores then load with plain contiguous DMAs.
- Dense pass: 16 unmasked 512-col chunks per row-tile; ACT exp(30*sim) reads
  PSUM directly, accumulating per-chunk sums into ncols[m][:,n]. No DVE pass.
- Own-block correction: the 1-2 own chunks containing the window are
  recomputed from ft_own with the host 0/1 mask (positives underflow to 0);
  their sums REPLACE the own columns of ncols via a dynamic-offset
  (partition-id) DVE copy — no large-magnitude cancellation anywhere.
- Band pass (positives): static [128,256] window matmul from ft_own + mask
  gives sum(exp(9 - 30*sim)); loss = ln(1 + P*N) per row; host averages.
- repeat>1 builds a timing NEFF with `repeat` back-to-back executions of the
  identical body (per-iteration AllGather output buffers — a Shared DRAM
  tensor may only have one writer instruction).
"""
import numpy as np
import ml_dtypes

import concourse.bass as bass
import concourse.bacc as bacc
import concourse.mybir as mybir
import concourse.tile as tile
from concourse.bass_utils import run_bass_kernel_spmd

F32 = mybir.dt.float32
BF16 = mybir.dt.bfloat16
AF = mybir.ActivationFunctionType
ALU = mybir.AluOpType

N, D, NCORES = 8192, 512, 8
R = N // NCORES  # rows per core
MT = R // 128  # row-tiles per core
NCH = N // 512  # dense 512-wide chunks
W = 256  # band window width
WOFF = [0, 64, 192, 320, 448, 576, 704, 768]  # static window start per tile
OWNMASK = {0: [0], 1: [0], 2: [0], 3: [0, 1], 4: [0, 1], 5: [1], 6: [1], 7: [1]}

_CACHED = {}


def _build_nc(repeat: int = 1):
    nc = bacc.Bacc("TRN2", target_bir_lowering=False, debug=False, num_devices=NCORES)

    feat_in = nc.dram_tensor("feat_in", [R, D], F32, kind="ExternalInput").ap()
    mask_in = nc.dram_tensor("mask_in", [R, R], BF16, kind="ExternalInput").ap()
    ident_in = nc.dram_tensor("ident_in", [128, 128], BF16, kind="ExternalInput").ap()
    o_loss = nc.dram_tensor("o_loss", [128, MT], F32, kind="ExternalOutput").ap()

    with tile.TileContext(nc) as tc:
        with (
            tc.tile_pool(name="io", bufs=3) as io,
            tc.tile_pool(name="fbp", bufs=3) as fbp,
            tc.tile_pool(name="stats", bufs=8) as stats,
            tc.tile_pool(name="singles", bufs=1) as singles,
            tc.tile_pool(name="maskp", bufs=2) as maskp,
            tc.tile_pool(name="ep", bufs=3) as epool,
            tc.tile_pool(name="bsmall", bufs=2) as bsmall,
            tc.tile_pool(name="psmain", bufs=5, space="PSUM") as psmain,
            tc.tile_pool(name="psband", bufs=1, space="PSUM") as psband,
            tc.tile_pool(name="pstp", bufs=2, space="PSUM") as pstp,
            tc.tile_pool(name="dram", bufs=1, space="DRAM") as dram,
        ):
            cc_in = dram.tile([D, R], BF16)
            cc_outs = [
                dram.tile([NCORES * D, R], BF16, addr_space="Shared", name=f"cc_out{r}")
                for r in range(repeat)
            ]

            ident = singles.tile([128, 128], BF16, name="ident")
            nc.sync.dma_start(out=ident, in_=ident_in)
            bias150 = singles.tile([128, 1], F32, name="bias150")
            nc.vector.memset(bias150, -150.0)

            losses = singles.tile([128, MT], F32, name="losses")
            ft_own = [singles.tile([128, R], BF16, name=f"ft_own{k}") for k in range(4)]
            ftall = [singles.tile([128, N], BF16, name=f"ftall{k}") for k in range(4)]
            ncols = [singles.tile([128, NCH], F32, name=f"ncols{m}") for m in range(MT)]
            pcols = singles.tile([128, MT], F32, name="pcols")
            ownm = [
                singles.tile([128, len(OWNMASK[m])], F32, name=f"ownm{m}")
                for m in range(MT)
            ]

            pid_dve = nc.vector.partition_id()

            for rep in range(repeat):
                cc_out = cc_outs[rep]

                # ---- Phase A: normalize own rows, PE-transpose to ft_own ----
                for m in range(MT):
                    x = io.tile([128, D], F32, tag="x")
                    nc.sync.dma_start(out=x, in_=feat_in[bass.ts(m, 128), :])
                    scr = io.tile([128, D], F32, tag="scr")
                    ss = stats.tile([128, 1], F32, tag="ss")
                    nc.scalar.activation(scr, x, AF.Square, accum_out=ss)
                    ssc = stats.tile([128, 1], F32, tag="ssc")
                    nc.vector.tensor_scalar_max(ssc, ss, 1e-16)
                    lnss = stats.tile([128, 1], F32, tag="lnss")
                    nc.scalar.activation(lnss, ssc, AF.Ln)
                    rinv = stats.tile([128, 1], F32, tag="rinv")
                    nc.scalar.activation(rinv, lnss, AF.Exp, scale=-0.5)
                    fb = fbp.tile([128, D], BF16, tag="fb")
                    nc.vector.tensor_scalar_mul(fb, x, rinv)
                    for k in range(4):
                        tp = pstp.tile([128, 128], BF16, tag="tp")
                        nc.tensor.transpose(tp, fb[:, bass.ts(k, 128)], ident)
                        nc.vector.tensor_copy(ft_own[k][:, bass.ts(m, 128)], tp)

                # ---- stage transposed block, AllGather ----
                for k in range(4):
                    nc.sync.dma_start(out=cc_in[bass.ts(k, 128), :], in_=ft_own[k])
                nc.gpsimd.collective_compute(
                    "AllGather",
                    ALU.bypass,
                    replica_groups=[list(range(NCORES))],
                    ins=[cc_in.opt()],
                    outs=[cc_out.opt()],
                )

                # ---- C0: band + masked own chunks (overlaps AllGather) ----
                for m in range(MT):
                    mt_t = maskp.tile([128, R], BF16, tag="mask")
                    nc.sync.dma_start(out=mt_t, in_=mask_in[bass.ts(m, 128), :])

                    bp = psband.tile([128, W], F32, tag="bps")
                    for k in range(4):
                        nc.tensor.matmul(
                            bp,
                            ft_own[k][:, bass.ts(m, 128)],
                            ft_own[k][:, WOFF[m] : WOFF[m] + W],
                            start=(k == 0),
                            stop=(k == 3),
                        )
                    ub = bsmall.tile([128, W], F32, tag="ub")
                    nc.vector.scalar_tensor_tensor(
                        ub,
                        in0=mt_t[:, WOFF[m] : WOFF[m] + W],
                        scalar=5.3,
                        in1=bp,
                        op0=ALU.mult,
                        op1=ALU.subtract,
                    )
                    eb = bsmall.tile([128, W], F32, tag="eb")
                    nc.scalar.activation(
                        eb,
                        ub,
                        AF.Exp,
                        scale=30.0,
                        bias=bias150,
                        accum_out=pcols[:, m : m + 1],
                    )

                    for j, n in enumerate(OWNMASK[m]):
                        po = psmain.tile([128, 512], F32, tag="ps", name=f"po{m}_{n}")
                        for k in range(4):
                            nc.tensor.matmul(
                                po,
                                ft_own[k][:, bass.ts(m, 128)],
                                ft_own[k][:, bass.ts(n, 512)],
                                start=(k == 0),
                                stop=(k == 3),
                            )
                        uo = epool.tile([128, 512], F32, tag="uo")
                        nc.vector.scalar_tensor_tensor(
                            uo,
                            in0=mt_t[:, bass.ts(n, 512)],
                            scalar=-33.333333,
                            in1=po,
                            op0=ALU.mult,
                            op1=ALU.add,
                        )
                        eo = epool.tile([128, 512], F32, tag="eo")
                        nc.scalar.activation(
                            eo,
                            uo,
                            AF.Exp,
                            scale=30.0,
                            accum_out=ownm[m][:, j : j + 1],
                        )

                # ---- load fT of all cores (plain contiguous DMAs) ----
                for k in range(4):
                    for rr in range(NCORES):
                        nc.sync.dma_start(
                            out=ftall[k][:, bass.ts(rr, R)],
                            in_=cc_out[512 * rr + 128 * k : 512 * rr + 128 * (k + 1), :],
                        )

                # ---- dense pass: 16 unmasked chunks per row-tile ----
                CHUNK_GROUPS = [
                    list(range(0, 5)),
                    list(range(5, 10)),
                    list(range(10, 15)),
                    [15],
                ]
                for m in range(MT):
                    psums = {}
                    for grp in CHUNK_GROUPS:
                        for k in range(4):
                            for n in grp:
                                if k == 0:
                                    psums[n] = psmain.tile(
                                        [128, 512], F32, tag="ps", name=f"ps{n}"
                                    )
                                nc.tensor.matmul(
                                    psums[n],
                                    ft_own[k][:, bass.ts(m, 128)],
                                    ftall[k][:, bass.ts(n, 512)],
                                    start=(k == 0),
                                    stop=(k == 3),
                                )
                        for n in grp:
                            e = epool.tile([128, 512], F32, tag="eo")
                            nc.scalar.activation(
                                e,
                                psums[n],
                                AF.Exp,
                                scale=30.0,
                                accum_out=ncols[m][:, n : n + 1],
                            )

                    # replace own (masked) chunk columns — dynamic offset 2*pid+j0
                    j0 = OWNMASK[m][0]
                    nc.vector.tensor_copy(
                        ncols[m][:, bass.ds(pid_dve * 2 + j0, len(OWNMASK[m]))],
                        ownm[m],
                    )

                    # combine: loss = ln(1 + P*N)
                    ncol = stats.tile([128, 1], F32, tag="ncol")
                    nc.vector.reduce_sum(ncol, ncols[m], axis=mybir.AxisListType.X)
                    lp = stats.tile([128, 1], F32, tag="lp")
                    nc.scalar.activation(lp, pcols[:, m : m + 1], AF.Ln)
                    lnn = stats.tile([128, 1], F32, tag="lnn")
                    nc.scalar.activation(lnn, ncol, AF.Ln)
                    xr = stats.tile([128, 1], F32, tag="xr")
                    nc.vector.tensor_tensor(xr, lp, lnn, op=ALU.add)
                    er = stats.tile([128, 1], F32, tag="er")
                    nc.scalar.activation(er, xr, AF.Exp)
                    er1 = stats.tile([128, 1], F32, tag="er1")
                    nc.vector.tensor_scalar_add(er1, er, 1.0)
                    nc.scalar.activation(losses[:, m : m + 1], er1, AF.Ln)

                nc.sync.dma_start(out=o_loss, in_=losses)

    nc.compile()
    return nc


def _prep(feat: np.ndarray, label: np.ndarray):
    """Sort by label, bin-pack whole groups into 8 bins of exactly 1024 rows."""
    order = np.argsort(label, kind="stable")
    lab_sorted = np.asarray(label)[order]
    _, starts, counts = np.unique(lab_sorted, return_index=True, return_counts=True)

    def pack(seed=None):
        idx = np.argsort(-counts, kind="stable")
        if seed is not None:
            rng = np.random.default_rng(seed)
            idx = idx.copy()
            sizes = counts[idx]
            for s in np.unique(sizes):
                sel = np.where(sizes == s)[0]
                idx[sel] = rng.permutation(idx[sel])
        bins = [[] for _ in range(NCORES)]
        rem = [R] * NCORES
        for g in idx:
            s = int(counts[g])
            for b in range(NCORES):
                if rem[b] >= s:
                    bins[b].append(g)
                    rem[b] -= s
                    break
            else:
                return None
        if any(r != 0 for r in rem):
            return None
        return bins

    bins = pack()
    if bins is None:
        for seed in range(200):
            bins = pack(seed)
            if bins is not None:
                break
    assert bins is not None, "group bin-packing failed"

    perm = np.concatenate(
        [
            np.concatenate([order[starts[g] : starts[g] + counts[g]] for g in b])
            for b in bins
        ]
    )
    feat_s = np.asarray(feat, dtype=np.float32)[perm]
    lab_s = np.asarray(label)[perm]

    ident = np.eye(128, dtype=ml_dtypes.bfloat16)
    in_maps = []
    for c in range(NCORES):
        lc = lab_s[c * R : (c + 1) * R]
        first_idx = {}
        last_idx = {}
        for i, l in enumerate(lc):
            if l not in first_idx:
                first_idx[l] = i
            last_idx[l] = i
        for i, l in enumerate(lc):
            m = i // 128
            w = WOFF[m]
            assert first_idx[l] >= w and last_idx[l] < w + W, (
                f"window violation core {c} row {i}"
            )
        mask = (lc[:, None] == lc[None, :]).astype(ml_dtypes.bfloat16)
        in_maps.append(
            {
                "feat_in": feat_s[c * R : (c + 1) * R],
                "mask_in": mask,
                "ident_in": ident,
            }
        )
    return in_maps


def kernel(feat: np.ndarray, label: np.ndarray) -> np.ndarray:
    feat = np.asarray(feat, dtype=np.float32)
    label = np.asarray(label)
    assert feat.shape == (N, D) and label.shape == (N,)

    in_maps = _prep(feat, label)

    if "nc" not in _CACHED:
        _CACHED["nc"] = _build_nc()
    nc = _CACHED["nc"]

    res = run_bass_kernel_spmd(nc, in_maps, core_ids=list(range(NCORES)))
    loss_rows = np.concatenate(
        [res.results[r]["o_loss"].T.reshape(-1) for r in range(NCORES)]
    )
    return np.float32(loss_rows.mean())
